# revision 8
# baseline (speedup 1.0000x reference)
"""MetaEmbedding classifier (retrieval_knn) — Trainium2 Bass kernel, 8-core data parallel.

Math (per batch row r, feat d in [0,2048), class c in [0,1000)):
  S1 = x @ centroids.T;  M[r] = max_c (S1 - c_sq/2);  min_d2 = x_sq - 2M
  reach = 10 / sqrt(min_d2)
  expH = exp(x @ W_hall.T + b_hall)  (softmax w/o max-sub; logits are O(1))
  rinv = 1 / sum_c expH
  mem[d, r] = sum_c centroids[c, d] * expH[c, r]
  sel = tanh(x @ W_sel.T + b_sel)
  infused = sel * mem * rinv                      (output 2)
  h2 = x + infused;  nsq = ||h2||^2
  g = 16 * reach / (1 + reach * sqrt(nsq))
  logits = g * (h2 @ (W_cos/|W_cos|).T)           (output 1)
  direct_feature = x                              (output 3)

Device layout is fully transposed ([feat/class on partitions, batch rows on the
free dim]) so no on-chip transposes are needed; the host pre-transposes x and
the weights, and transposes the outputs back. All matmuls run fp32r (full PE
rate at N>=256, ~1e-4 matmul rel err). Batch is sharded 8 ways (1024 rows/core);
weights are replicated. h2 is spilled to HBM between the fusion pipeline and the
classifier matmul because SBUF can't hold xT + expH + h2 at once.
"""
import sys

sys.path.insert(0, "/opt/trn_rl_repo")

import numpy as np

import concourse.bacc as bacc
import concourse.mybir as mybir
import concourse.tile as tile
from concourse.bass_utils import run_bass_kernel_spmd

f32 = mybir.dt.float32
f32r = mybir.dt.float32r
AF = mybir.ActivationFunctionType
ALU = mybir.AluOpType

D = 2048          # feature dim
C = 1000          # classes
B = 1024          # rows per core (8192 / 8)
KT = D // 128     # 16 k-tiles over feature dim
MT = D // 128     # 16 m-tiles over output feature dim
CT = 8            # class tiles of 125
CP = C // CT      # 125
RT = B // 128     # 8 row-tiles (phase 1)
RC = B // 512     # 2 row chunks of 512 (moving-operand max for 4-byte dtypes)
RW = 512
CC = 2            # class chunks of 500 (phase 1 moving operand)
CCH = C // CC     # 500

_CACHE = {}


def _build_program():
    nc = bacc.Bacc("TRN2", target_bir_lowering=False, debug=False,
                   enable_asserts=True, num_devices=8)

    XT = nc.dram_tensor("xT", [D, B], f32r, kind="ExternalInput").ap()
    CENTT = nc.dram_tensor("centT", [D, C], f32r, kind="ExternalInput").ap()
    CENTN = nc.dram_tensor("centN", [C, D], f32r, kind="ExternalInput").ap()
    WHT = nc.dram_tensor("whallT", [D, C], f32r, kind="ExternalInput").ap()
    WST = nc.dram_tensor("wselT", [D, D], f32r, kind="ExternalInput").ap()
    WCT = nc.dram_tensor("wcosnT", [D, C], f32r, kind="ExternalInput").ap()
    BH = nc.dram_tensor("bhall", [C, 1], f32, kind="ExternalInput").ap()
    BS = nc.dram_tensor("bsel", [D, 1], f32, kind="ExternalInput").ap()
    CSQH = nc.dram_tensor("csqh", [1, C], f32r, kind="ExternalInput").ap()
    ONESC = nc.dram_tensor("onesc", [128, 1], f32r, kind="ExternalInput").ap()
    ONESR = nc.dram_tensor("onesr", [1, 128], f32r, kind="ExternalInput").ap()
    LG = nc.dram_tensor("logitsT", [C, B], f32, kind="ExternalOutput").ap()
    DBG = nc.dram_tensor("dbg", [6, B], f32, kind="ExternalOutput").ap()
    DBG2 = nc.dram_tensor("dbg2", [128, 8], f32, kind="ExternalOutput").ap()
    DBG3 = nc.dram_tensor("dbg3", [2, C], f32, kind="ExternalOutput").ap()
    INF = nc.dram_tensor("infusedT", [D, B], f32, kind="ExternalOutput").ap()

    with tile.TileContext(nc) as tc:
        with tc.tile_pool(name="persist", bufs=1) as pp, \
             tc.tile_pool(name="vecs", bufs=1) as vp, \
             tc.tile_pool(name="dram", bufs=1, space="DRAM") as dp:
            # ---- persistent SBUF state ----
            xt = pp.tile([128, KT * B], f32r, tag="xt")            # 64KB/part
            expht = pp.tile([CP, CT * B], f32r, tag="expht")       # 32KB/part
            rinv_b = pp.tile([128, B], f32, tag="rinv_b")          # 4KB
            g_b = pp.tile([128, B], f32, tag="g_b")                # 4KB
            onesc = pp.tile([128, 1], f32r, tag="onesc")
            onesr = pp.tile([1, 128], f32r, tag="onesr")
            bh = pp.tile([CP, CT], f32, tag="bh")
            bs = pp.tile([128, MT], f32, tag="bs")
            reach = vp.tile([1, B], f32, tag="reach")
            xsqv = vp.tile([1, B], f32, tag="xsqv")
            m1row = vp.tile([1, B], f32, tag="m1row")
            rinv_v = vp.tile([1, B], f32r, tag="rinv_v")
            g_v = vp.tile([1, B], f32r, tag="g_v")
            vt0 = vp.tile([1, B], f32, tag="vt0")
            vt1 = vp.tile([1, B], f32, tag="vt1")

            h2s = dp.tile([D, B], f32r, tag="h2s")                 # HBM spill
            m1d = dp.tile([1, B], f32, tag="m1d")                  # transpose bounce

            for k in range(KT):
                nc.sync.dma_start(xt[:, k * B:(k + 1) * B],
                                  XT[k * 128:(k + 1) * 128, :])
            nc.sync.dma_start(onesc[:], ONESC)
            nc.sync.dma_start(onesr[:], ONESR)
            nc.sync.dma_start(bh[:], BH[:, 0].rearrange("(t p) -> p t", p=CP))
            nc.sync.dma_start(bs[:], BS[:, 0].rearrange("(m p) -> p m", p=128))

            csq_b = pp.tile([128, C], f32, tag="csq_b")

            # ---- P0: x_sq column sums + c_sq broadcast ----
            with tc.tile_pool(name="p0", bufs=1) as p0, \
                 tc.tile_pool(name="p0ps", bufs=1, space="PSUM") as p0ps:
                csqh_sb = p0.tile([1, C], f32r, tag="csqh")
                nc.sync.dma_start(csqh_sb[:], CSQH)
                for cc in range(CC):
                    cb_ps = p0ps.tile([128, CCH], f32, tag="cb", bufs=2)
                    nc.tensor.matmul(cb_ps[:], onesr[:],
                                     csqh_sb[:, cc * CCH:(cc + 1) * CCH],
                                     start=True, stop=True)
                    nc.vector.tensor_copy(csq_b[:, cc * CCH:(cc + 1) * CCH], cb_ps[:])

                xsq_ps = [p0ps.tile([1, RW], f32, tag=f"xsq{rc}", name=f"xsq{rc}") for rc in range(RC)]
                for m in range(MT):
                    sqx = p0.tile([128, B], f32r, tag="sqx", bufs=2)
                    nc.scalar.activation(sqx[:], xt[:, m * B:(m + 1) * B], AF.Square)
                    for rc in range(RC):
                        nc.tensor.matmul(xsq_ps[rc][:], onesc[:],
                                         sqx[:, rc * RW:(rc + 1) * RW],
                                         start=(m == 0), stop=(m == MT - 1))
                for rc in range(RC):
                    nc.vector.tensor_copy(xsqv[:, rc * RW:(rc + 1) * RW],
                                          xsq_ps[rc][:])

            # ---- P1: min distance to centroids -> reach ----
            with tc.tile_pool(name="p1", bufs=1) as p1, \
                 tc.tile_pool(name="p1ps", bufs=1, space="PSUM") as p1ps:
                s1_ps = [p1ps.tile([128, CCH], f32, tag=f"s1_{t}", name=f"s1_{t}") for t in range(RT)]
                mredcc = [p1.tile([128, RT], f32, tag=f"mred{cc}", name=f"mred{cc}") for cc in range(CC)]
                for cc in range(CC):
                    for k in range(KT):
                        centk = p1.tile([128, CCH], f32r, tag="centk", bufs=3)
                        nc.sync.dma_start(
                            centk[:],
                            CENTT[k * 128:(k + 1) * 128, cc * CCH:(cc + 1) * CCH])
                        for t in range(RT):
                            nc.tensor.matmul(
                                s1_ps[t][:],
                                xt[:, k * B + t * 128: k * B + (t + 1) * 128],
                                centk[:],
                                start=(k == 0), stop=(k == KT - 1))
                    for t in range(RT):
                        u = p1.tile([128, CCH], f32, tag="u", bufs=2)
                        nc.vector.tensor_tensor(u[:], s1_ps[t][:],
                                                csq_b[:, cc * CCH:(cc + 1) * CCH],
                                                op=ALU.subtract)
                        nc.vector.reduce_max(mredcc[cc][:, t:t + 1], u[:],
                                             axis=mybir.AxisListType.X)
                m1cols = p1.tile([128, RT], f32, tag="m1cols")
                nc.vector.tensor_tensor(m1cols[:], mredcc[0][:], mredcc[1][:],
                                        op=ALU.max)
                nc.sync.dma_start(DBG2, m1cols[:])
                nc.sync.dma_start(DBG3, csq_b[0:2, :])
                # transpose [128, RT] -> [1, B] (r = t*128 + p) via HBM bounce:
                # DRAM-side AP does the permutation; SBUF APs stay natural.
                nc.sync.dma_start(m1d[0, :].rearrange("(t p) -> p t", p=128),
                                  m1cols[:])
                nc.sync.dma_start(m1row[:], m1d[:])
                # reach = 10 / sqrt(max(xsq - 2*m1, eps))
                nc.vector.scalar_tensor_tensor(vt0[:], m1row[:], -2.0, xsqv[:],
                                               op0=ALU.mult, op1=ALU.add)
                nc.vector.tensor_scalar_max(vt0[:], vt0[:], 1e-12)
                nc.scalar.activation(vt1[:], vt0[:], AF.Sqrt)
                nc.vector.reciprocal(vt0[:], vt1[:])
                nc.vector.tensor_scalar_mul(reach[:], vt0[:], 10.0)

            # ---- P2: hallucination attention expH + softmax denominator ----
            with tc.tile_pool(name="p2", bufs=1) as p2, \
                 tc.tile_pool(name="p2ps", bufs=1, space="PSUM") as p2ps:
                for ct in range(CT):
                    h_ps = [p2ps.tile([CP, RW], f32, tag=f"h{rc}", bufs=2, name=f"h{rc}")
                            for rc in range(RC)]
                    for k in range(KT):
                        whk = p2.tile([128, CP], f32r, tag="whk", bufs=4)
                        nc.sync.dma_start(
                            whk[:],
                            WHT[k * 128:(k + 1) * 128, ct * CP:(ct + 1) * CP])
                        for rc in range(RC):
                            nc.tensor.matmul(
                                h_ps[rc][:], whk[:],
                                xt[:, k * B + rc * RW: k * B + rc * RW + RW],
                                start=(k == 0), stop=(k == KT - 1))
                    for rc in range(RC):
                        nc.scalar.activation(
                            expht[:, ct * B + rc * RW: ct * B + rc * RW + RW],
                            h_ps[rc][:], AF.Exp, bias=bh[:, ct:ct + 1])
                s_ps = [p2ps.tile([1, RW], f32, tag=f"s{rc}", name=f"s{rc}") for rc in range(RC)]
                for rc in range(RC):
                    for ct in range(CT):
                        nc.tensor.matmul(
                            s_ps[rc][:], onesc[0:CP, :],
                            expht[:, ct * B + rc * RW: ct * B + rc * RW + RW],
                            start=(ct == 0), stop=(ct == CT - 1))
                with nc.allow_low_precision(reason="f32r vec for bcast matmul"):
                    for rc in range(RC):
                        nc.vector.reciprocal(rinv_v[:, rc * RW:(rc + 1) * RW],
                                             s_ps[rc][:])
                for rc in range(RC):
                    rb_ps = p2ps.tile([128, RW], f32, tag="rb", bufs=2)
                    nc.tensor.matmul(rb_ps[:], onesr[:],
                                     rinv_v[:, rc * RW:(rc + 1) * RW],
                                     start=True, stop=True)
                    nc.vector.tensor_copy(rinv_b[:, rc * RW:(rc + 1) * RW],
                                          rb_ps[:])

            # ---- P3: selector + memory + fusion pipeline over feature m-tiles ----
            with tc.tile_pool(name="p3", bufs=1) as p3, \
                 tc.tile_pool(name="p3nsq", bufs=1, space="PSUM") as p3nsq, \
                 tc.tile_pool(name="p3ps", bufs=1, space="PSUM") as p3ps:
                nsq_ps = [p3nsq.tile([1, RW], f32, tag=f"nsq{rc}", name=f"nsq{rc}") for rc in range(RC)]
                for m in range(MT):
                    wselblk = p3.tile([128, KT * 128], f32r, tag="wselblk", bufs=2)
                    nc.sync.dma_start(
                        wselblk[:].rearrange("p (k c) -> p k c", k=KT),
                        WST[:, m * 128:(m + 1) * 128]
                        .rearrange("(k p) c -> p k c", p=128))
                    centblk = p3.tile([CP, CT * 128], f32r, tag="centblk", bufs=2)
                    nc.sync.dma_start(
                        centblk[:].rearrange("p (t c) -> p t c", t=CT),
                        CENTN[:, m * 128:(m + 1) * 128]
                        .rearrange("(t p) c -> p t c", p=CP))
                    for rc in range(RC):
                        sel_ps = p3ps.tile([128, RW], f32, tag="sel", bufs=2)
                        for k in range(KT):
                            nc.tensor.matmul(
                                sel_ps[:],
                                wselblk[:, k * 128:(k + 1) * 128],
                                xt[:, k * B + rc * RW: k * B + rc * RW + RW],
                                start=(k == 0), stop=(k == KT - 1))
                        mem_ps = p3ps.tile([128, RW], f32, tag="mem", bufs=2)
                        for ct in range(CT):
                            nc.tensor.matmul(
                                mem_ps[:],
                                centblk[:, ct * 128:(ct + 1) * 128],
                                expht[:, ct * B + rc * RW: ct * B + rc * RW + RW],
                                start=(ct == 0), stop=(ct == CT - 1))
                        selt = p3.tile([128, RW], f32, tag="selt", bufs=2)
                        nc.scalar.activation(selt[:], sel_ps[:], AF.Tanh,
                                             bias=bs[:, m:m + 1])
                        t0 = p3.tile([128, RW], f32, tag="t0", bufs=2)
                        nc.vector.tensor_tensor(t0[:], mem_ps[:], selt[:], op=ALU.mult)
                        inf = p3.tile([128, RW], f32, tag="inf", bufs=3)
                        nc.vector.tensor_tensor(
                            inf[:], t0[:], rinv_b[:, rc * RW:(rc + 1) * RW],
                            op=ALU.mult)
                        nc.sync.dma_start(
                            INF[m * 128:(m + 1) * 128, rc * RW:(rc + 1) * RW],
                            inf[:])
                        h2 = p3.tile([128, RW], f32r, tag="h2", bufs=3)
                        nc.vector.tensor_tensor(
                            h2[:].bitcast(f32), inf[:],
                            xt[:, m * B + rc * RW: m * B + rc * RW + RW].bitcast(f32),
                            op=ALU.add)
                        nc.sync.dma_start(
                            h2s[m * 128:(m + 1) * 128, rc * RW:(rc + 1) * RW],
                            h2[:])
                        sq = p3.tile([128, RW], f32r, tag="sq", bufs=2)
                        nc.scalar.activation(sq[:], h2[:], AF.Square)
                        nc.tensor.matmul(nsq_ps[rc][:], onesc[:], sq[:],
                                         start=(m == 0), stop=(m == MT - 1))

                # g = 16*reach / (1 + reach*sqrt(nsq))
                for rc in range(RC):
                    nc.scalar.activation(vt0[:, rc * RW:(rc + 1) * RW],
                                         nsq_ps[rc][:], AF.Sqrt)
                nc.vector.tensor_tensor(vt0[:], vt0[:], reach[:], op=ALU.mult)
                nc.vector.tensor_scalar_add(vt0[:], vt0[:], 1.0)
                nc.vector.reciprocal(vt1[:], vt0[:])
                nc.vector.tensor_tensor(vt1[:], vt1[:], reach[:], op=ALU.mult)
                with nc.allow_low_precision(reason="f32r vec for bcast matmul"):
                    nc.vector.tensor_scalar_mul(g_v[:], vt1[:], 16.0)
                nc.sync.dma_start(DBG[0:1, :], reach[:])
                nc.sync.dma_start(DBG[1:2, :], xsqv[:])
                nc.sync.dma_start(DBG[2:3, :], m1row[:])
                nc.sync.dma_start(DBG[3:4, :], g_v[:].bitcast(f32))
                nc.sync.dma_start(DBG[4:5, :], rinv_v[:].bitcast(f32))
                nc.sync.dma_start(DBG[5:6, :], vt1[:])
                with tc.tile_pool(name="gps", bufs=1, space="PSUM") as gps:
                    for rc in range(RC):
                        gb_ps = gps.tile([128, RW], f32, tag="gb", bufs=2)
                        nc.tensor.matmul(gb_ps[:], onesr[:],
                                         g_v[:, rc * RW:(rc + 1) * RW],
                                         start=True, stop=True)
                        nc.vector.tensor_copy(g_b[:, rc * RW:(rc + 1) * RW],
                                              gb_ps[:])

            # ---- P4: cos-norm classifier ----
            with tc.tile_pool(name="p4", bufs=1) as p4, \
                 tc.tile_pool(name="p4ps", bufs=1, space="PSUM") as p4ps:
                for rc in range(RC):
                    lg_ps = [p4ps.tile([CP, RW], f32, tag=f"lg{ct}", name=f"lg{ct}")
                             for ct in range(CT)]
                    for k in range(KT):
                        wcosk = p4.tile([128, C], f32r, tag="wcosk", bufs=3)
                        nc.sync.dma_start(wcosk[:], WCT[k * 128:(k + 1) * 128, :])
                        h2k = p4.tile([128, RW], f32r, tag="h2k", bufs=3)
                        nc.sync.dma_start(
                            h2k[:],
                            h2s[k * 128:(k + 1) * 128, rc * RW:(rc + 1) * RW])
                        for ct in range(CT):
                            nc.tensor.matmul(
                                lg_ps[ct][:],
                                wcosk[:, ct * CP:(ct + 1) * CP],
                                h2k[:],
                                start=(k == 0), stop=(k == KT - 1))
                    for ct in range(CT):
                        lgout = p4.tile([CP, RW], f32, tag="lgout", bufs=3)
                        nc.vector.tensor_tensor(
                            lgout[:], lg_ps[ct][:],
                            g_b[0:CP, rc * RW:(rc + 1) * RW], op=ALU.mult)
                        nc.sync.dma_start(
                            LG[ct * CP:(ct + 1) * CP, rc * RW:(rc + 1) * RW],
                            lgout[:])

    nc.compile()
    return nc


def kernel(x, centroids, W_hall, b_hall, W_sel, b_sel, W_cos):
    x = np.asarray(x, np.float32)
    centroids = np.asarray(centroids, np.float32)
    W_hall = np.asarray(W_hall, np.float32)
    b_hall = np.asarray(b_hall, np.float32)
    W_sel = np.asarray(W_sel, np.float32)
    b_sel = np.asarray(b_sel, np.float32)
    W_cos = np.asarray(W_cos, np.float32)

    if "nc" not in _CACHE:
        _CACHE["nc"] = _build_program()
    nc = _CACHE["nc"]

    NB = x.shape[0]
    ncores = 8
    shard = NB // ncores

    wnorm = W_cos / np.linalg.norm(W_cos, axis=1, keepdims=True)
    shared = {
        "centT": np.ascontiguousarray(centroids.T),
        "centN": np.ascontiguousarray(centroids),
        "whallT": np.ascontiguousarray(W_hall.T),
        "wselT": np.ascontiguousarray(W_sel.T),
        "wcosnT": np.ascontiguousarray(wnorm.T),
        "bhall": np.ascontiguousarray(b_hall.reshape(C, 1)),
        "bsel": np.ascontiguousarray(b_sel.reshape(D, 1)),
        "csqh": np.ascontiguousarray((np.sum(centroids * centroids, axis=1) / 2.0)
                                     .reshape(1, C)),
        "onesc": np.ones((128, 1), np.float32),
        "onesr": np.ones((1, 128), np.float32),
    }
    in_maps = []
    for i in range(ncores):
        m = dict(shared)
        m["xT"] = np.ascontiguousarray(x[i * shard:(i + 1) * shard].T)
        in_maps.append(m)

    res = run_bass_kernel_spmd(nc, in_maps, list(range(ncores)), **_CACHE.get("run_kwargs", {}))
    _CACHE["last_result"] = res

    logits = np.concatenate(
        [np.ascontiguousarray(res.results[i]["logitsT"].T) for i in range(ncores)],
        axis=0)
    infused = np.concatenate(
        [np.ascontiguousarray(res.results[i]["infusedT"].T) for i in range(ncores)],
        axis=0)
    return logits, x, infused


# revision 11
# speedup vs baseline: 1.0018x; 1.0018x over previous
"""MetaEmbedding classifier (retrieval_knn) — Trainium2 Bass kernel, 8-core data parallel.

Math (per batch row r, feat d in [0,2048), class c in [0,1000)):
  S1 = x @ centroids.T;  M[r] = max_c (S1 - c_sq/2);  min_d2 = x_sq - 2M
  reach = 10 / sqrt(min_d2)
  expH = exp(x @ W_hall.T + b_hall)  (softmax w/o max-sub; logits are O(1))
  rinv = 1 / sum_c expH
  mem[d, r] = sum_c centroids[c, d] * expH[c, r]
  sel = tanh(x @ W_sel.T + b_sel)
  infused = sel * mem * rinv                      (output 2)
  h2 = x + infused;  nsq = ||h2||^2
  g = 16 * reach / (1 + reach * sqrt(nsq))
  logits = g * (h2 @ (W_cos/|W_cos|).T)           (output 1)
  direct_feature = x                              (output 3)

Device layout is fully transposed ([feat/class on partitions, batch rows on the
free dim]) so no on-chip transposes are needed; the host pre-transposes x and
the weights, and transposes the outputs back. All matmuls run fp32r (full PE
rate at N>=256, ~1e-4 matmul rel err). Batch is sharded 8 ways (1024 rows/core);
weights are replicated. h2 is spilled to HBM between the fusion pipeline and the
classifier matmul because SBUF can't hold xT + expH + h2 at once.
"""
import sys

sys.path.insert(0, "/opt/trn_rl_repo")

import numpy as np

import concourse.bacc as bacc
import concourse.mybir as mybir
import concourse.tile as tile
from concourse.bass_utils import run_bass_kernel_spmd

f32 = mybir.dt.float32
f32r = mybir.dt.float32r
AF = mybir.ActivationFunctionType
ALU = mybir.AluOpType

D = 2048          # feature dim
C = 1000          # classes
B = 1024          # rows per core (8192 / 8)
KT = D // 128     # 16 k-tiles over feature dim
MT = D // 128     # 16 m-tiles over output feature dim
CT = 8            # class tiles of 125
CP = C // CT      # 125
RT = B // 128     # 8 row-tiles (phase 1)
RC = B // 512     # 2 row chunks of 512 (moving-operand max for 4-byte dtypes)
RW = 512
CC = 2            # class chunks of 500 (phase 1 moving operand)
CCH = C // CC     # 500

_CACHE = {}


def _build_program():
    nc = bacc.Bacc("TRN2", target_bir_lowering=False, debug=False,
                   enable_asserts=True, num_devices=8)

    XT = nc.dram_tensor("xT", [D, B], f32r, kind="ExternalInput").ap()
    CENTT = nc.dram_tensor("centT", [D, C], f32r, kind="ExternalInput").ap()
    CENTN = nc.dram_tensor("centN", [C, D], f32r, kind="ExternalInput").ap()
    WHT = nc.dram_tensor("whallT", [D, C], f32r, kind="ExternalInput").ap()
    WST = nc.dram_tensor("wselT", [D, D], f32r, kind="ExternalInput").ap()
    WCT = nc.dram_tensor("wcosnT", [D, C], f32r, kind="ExternalInput").ap()
    BH = nc.dram_tensor("bhall", [C, 1], f32, kind="ExternalInput").ap()
    BS = nc.dram_tensor("bsel", [D, 1], f32, kind="ExternalInput").ap()
    NCSQ = nc.dram_tensor("ncsq", [1, C], f32r, kind="ExternalInput").ap()
    ONESC = nc.dram_tensor("onesc", [128, 1], f32r, kind="ExternalInput").ap()
    ONESR = nc.dram_tensor("onesr", [1, 128], f32r, kind="ExternalInput").ap()
    LG = nc.dram_tensor("logitsT", [C, B], f32, kind="ExternalOutput").ap()
    INF = nc.dram_tensor("infusedT", [D, B], f32, kind="ExternalOutput").ap()

    with tile.TileContext(nc) as tc:
        with tc.tile_pool(name="persist", bufs=1) as pp, \
             tc.tile_pool(name="vecs", bufs=1) as vp, \
             tc.tile_pool(name="dram", bufs=1, space="DRAM") as dp:
            # ---- persistent SBUF state ----
            xt = pp.tile([128, KT * B], f32r, tag="xt")            # 64KB/part
            expht = pp.tile([CP, CT * B], f32r, tag="expht")       # 32KB/part
            rinv_b = pp.tile([128, B], f32, tag="rinv_b")          # 4KB
            g_b = pp.tile([128, B], f32, tag="g_b")                # 4KB
            onesc = pp.tile([128, 1], f32r, tag="onesc")
            onesr = pp.tile([1, 128], f32r, tag="onesr")
            bh = pp.tile([CP, CT], f32, tag="bh")
            bs = pp.tile([128, MT], f32, tag="bs")
            reach = vp.tile([1, B], f32, tag="reach")
            xsqv = vp.tile([1, B], f32, tag="xsqv")
            m1row = vp.tile([1, B], f32, tag="m1row")
            rinv_v = vp.tile([1, B], f32, tag="rinv_v")
            g_v = vp.tile([1, B], f32, tag="g_v")
            vt0 = vp.tile([1, B], f32, tag="vt0")
            vt1 = vp.tile([1, B], f32, tag="vt1")

            h2s = dp.tile([D, B], f32r, tag="h2s")                 # HBM spill
            m1d = dp.tile([1, B], f32, tag="m1d")                  # transpose bounce
            rinv_d = dp.tile([1, B], f32, tag="rinv_d")
            g_d = dp.tile([1, B], f32, tag="g_d")

            for k in range(KT):
                nc.sync.dma_start(xt[:, k * B:(k + 1) * B],
                                  XT[k * 128:(k + 1) * 128, :])
            nc.sync.dma_start(onesc[:], ONESC)
            nc.sync.dma_start(onesr[:], ONESR)
            nc.sync.dma_start(bh[:], BH[:, 0].rearrange("(t p) -> p t", p=CP))
            nc.sync.dma_start(bs[:], BS[:, 0].rearrange("(m p) -> p m", p=128))
            ncsq = pp.tile([1, C], f32r, tag="ncsq")
            nc.sync.dma_start(ncsq[:], NCSQ)

            # ---- P0: x_sq column sums + c_sq broadcast ----
            with tc.tile_pool(name="p0", bufs=1) as p0, \
                 tc.tile_pool(name="p0ps", bufs=1, space="PSUM") as p0ps:
                xsq_ps = [p0ps.tile([1, RW], f32, tag=f"xsq{rc}", name=f"xsq{rc}") for rc in range(RC)]
                for m in range(MT):
                    sqx = p0.tile([128, B], f32r, tag="sqx", bufs=2)
                    nc.scalar.activation(sqx[:], xt[:, m * B:(m + 1) * B], AF.Square)
                    for rc in range(RC):
                        nc.tensor.matmul(xsq_ps[rc][:], onesc[:],
                                         sqx[:, rc * RW:(rc + 1) * RW],
                                         start=(m == 0), stop=(m == MT - 1))
                for rc in range(RC):
                    nc.vector.tensor_copy(xsqv[:, rc * RW:(rc + 1) * RW],
                                          xsq_ps[rc][:])

            # ---- P1: min distance to centroids -> reach ----
            with tc.tile_pool(name="p1", bufs=1) as p1, \
                 tc.tile_pool(name="p1ps", bufs=1, space="PSUM") as p1ps:
                s1_ps = [p1ps.tile([128, CCH], f32, tag=f"s1_{t}", name=f"s1_{t}") for t in range(RT)]
                mredcc = [p1.tile([128, RT], f32, tag=f"mred{cc}", name=f"mred{cc}") for cc in range(CC)]
                for cc in range(CC):
                    # seed the accumulators with -csq/2 via a K=1 ones-matmul
                    for t in range(RT):
                        nc.tensor.matmul(s1_ps[t][:], onesr[:],
                                         ncsq[:, cc * CCH:(cc + 1) * CCH],
                                         start=True, stop=False)
                    for k in range(KT):
                        centk = p1.tile([128, CCH], f32r, tag="centk", bufs=4)
                        nc.sync.dma_start(
                            centk[:],
                            CENTT[k * 128:(k + 1) * 128, cc * CCH:(cc + 1) * CCH])
                        for t in range(RT):
                            nc.tensor.matmul(
                                s1_ps[t][:],
                                xt[:, k * B + t * 128: k * B + (t + 1) * 128],
                                centk[:],
                                start=False, stop=(k == KT - 1))
                    for t in range(RT):
                        nc.vector.reduce_max(mredcc[cc][:, t:t + 1], s1_ps[t][:],
                                             axis=mybir.AxisListType.X)
                m1cols = p1.tile([128, RT], f32, tag="m1cols")
                nc.vector.tensor_tensor(m1cols[:], mredcc[0][:], mredcc[1][:],
                                        op=ALU.max)
                # transpose [128, RT] -> [1, B] (r = t*128 + p) via HBM bounce:
                # DRAM-side AP does the permutation; SBUF APs stay natural.
                nc.sync.dma_start(m1d[0, :].rearrange("(t p) -> p t", p=128),
                                  m1cols[:])

            # reach chain runs outside the P1 pools so it doesn't block P2/P3
            nc.sync.dma_start(m1row[:], m1d[:])
            # reach = 10 / sqrt(max(xsq - 2*m1, eps))
            nc.vector.scalar_tensor_tensor(vt0[:], m1row[:], -2.0, xsqv[:],
                                           op0=ALU.mult, op1=ALU.add)
            nc.vector.tensor_scalar_max(vt0[:], vt0[:], 1e-12)
            nc.scalar.activation(vt1[:], vt0[:], AF.Sqrt)
            nc.vector.reciprocal(vt0[:], vt1[:])
            nc.vector.tensor_scalar_mul(reach[:], vt0[:], 10.0)

            # ---- P2: hallucination attention expH + softmax denominator ----
            with tc.tile_pool(name="p2", bufs=1) as p2, \
                 tc.tile_pool(name="p2ps", bufs=1, space="PSUM") as p2ps:
                for ct in range(CT):
                    h_ps = [p2ps.tile([CP, RW], f32, tag=f"h{rc}", bufs=2, name=f"h{rc}")
                            for rc in range(RC)]
                    for k in range(KT):
                        whk = p2.tile([128, CP], f32r, tag="whk", bufs=4)
                        nc.sync.dma_start(
                            whk[:],
                            WHT[k * 128:(k + 1) * 128, ct * CP:(ct + 1) * CP])
                        for rc in range(RC):
                            nc.tensor.matmul(
                                h_ps[rc][:], whk[:],
                                xt[:, k * B + rc * RW: k * B + rc * RW + RW],
                                start=(k == 0), stop=(k == KT - 1))
                    for rc in range(RC):
                        nc.scalar.activation(
                            expht[:, ct * B + rc * RW: ct * B + rc * RW + RW],
                            h_ps[rc][:], AF.Exp, bias=bh[:, ct:ct + 1])
                s_ps = [p2ps.tile([1, RW], f32, tag=f"s{rc}", name=f"s{rc}") for rc in range(RC)]
                for rc in range(RC):
                    for ct in range(CT):
                        nc.tensor.matmul(
                            s_ps[rc][:], onesc[0:CP, :],
                            expht[:, ct * B + rc * RW: ct * B + rc * RW + RW],
                            start=(ct == 0), stop=(ct == CT - 1))
                for rc in range(RC):
                    nc.vector.reciprocal(rinv_v[:, rc * RW:(rc + 1) * RW],
                                         s_ps[rc][:])

            # broadcast rinv across partitions via an HBM bounce + step-0 read
            nc.sync.dma_start(rinv_d[:], rinv_v[:])
            nc.sync.dma_start(rinv_b[:], rinv_d[0:1, :].to_broadcast((128, B)))

            # ---- P3: selector + memory + fusion pipeline over feature m-tiles ----
            with tc.tile_pool(name="p3", bufs=1) as p3, \
                 tc.tile_pool(name="p3nsq", bufs=1, space="PSUM") as p3nsq, \
                 tc.tile_pool(name="p3ps", bufs=1, space="PSUM") as p3ps:
                nsq_ps = [p3nsq.tile([1, RW], f32, tag=f"nsq{rc}", name=f"nsq{rc}") for rc in range(RC)]
                for m in range(MT):
                    wselblk = pp.tile([128, KT * 128], f32r, tag="wselblk", bufs=2)
                    nc.sync.dma_start(
                        wselblk[:].rearrange("p (k c) -> p k c", k=KT),
                        WST[:, m * 128:(m + 1) * 128]
                        .rearrange("(k p) c -> p k c", p=128))
                    centblk = pp.tile([CP, CT * 128], f32r, tag="centblk", bufs=2)
                    nc.sync.dma_start(
                        centblk[:].rearrange("p (t c) -> p t c", t=CT),
                        CENTN[:, m * 128:(m + 1) * 128]
                        .rearrange("(t p) c -> p t c", p=CP))
                    for rc in range(RC):
                        sel_ps = p3ps.tile([128, RW], f32, tag="sel", bufs=2)
                        for k in range(KT):
                            nc.tensor.matmul(
                                sel_ps[:],
                                wselblk[:, k * 128:(k + 1) * 128],
                                xt[:, k * B + rc * RW: k * B + rc * RW + RW],
                                start=(k == 0), stop=(k == KT - 1))
                        mem_ps = p3ps.tile([128, RW], f32, tag="mem", bufs=2)
                        for ct in range(CT):
                            nc.tensor.matmul(
                                mem_ps[:],
                                centblk[:, ct * 128:(ct + 1) * 128],
                                expht[:, ct * B + rc * RW: ct * B + rc * RW + RW],
                                start=(ct == 0), stop=(ct == CT - 1))
                        selt = p3.tile([128, RW], f32, tag="selt", bufs=2)
                        nc.scalar.activation(selt[:], sel_ps[:], AF.Tanh,
                                             bias=bs[:, m:m + 1])
                        t0 = p3.tile([128, RW], f32, tag="t0", bufs=2)
                        nc.vector.tensor_tensor(t0[:], mem_ps[:], selt[:], op=ALU.mult)
                        inf = p3.tile([128, RW], f32, tag="inf", bufs=3)
                        nc.vector.tensor_tensor(
                            inf[:], t0[:], rinv_b[:, rc * RW:(rc + 1) * RW],
                            op=ALU.mult)
                        nc.sync.dma_start(
                            INF[m * 128:(m + 1) * 128, rc * RW:(rc + 1) * RW],
                            inf[:])
                        h2 = p3.tile([128, RW], f32r, tag="h2", bufs=3)
                        nc.vector.tensor_tensor(
                            h2[:].bitcast(f32), inf[:],
                            xt[:, m * B + rc * RW: m * B + rc * RW + RW].bitcast(f32),
                            op=ALU.add)
                        nc.sync.dma_start(
                            h2s[m * 128:(m + 1) * 128, rc * RW:(rc + 1) * RW],
                            h2[:])
                        sq = p3.tile([128, RW], f32r, tag="sq", bufs=2)
                        nc.scalar.activation(sq[:], h2[:], AF.Square)
                        nc.tensor.matmul(nsq_ps[rc][:], onesc[:], sq[:],
                                         start=(m == 0), stop=(m == MT - 1))

                # free the nsq PSUM banks immediately; sqrt from SBUF later
                for rc in range(RC):
                    nc.scalar.activation(vt0[:, rc * RW:(rc + 1) * RW],
                                         nsq_ps[rc][:], AF.Sqrt)

            # g = 16*reach / (1 + reach*sqrt(nsq)) — outside P3 pools
            nc.vector.tensor_tensor(vt0[:], vt0[:], reach[:], op=ALU.mult)
            nc.vector.tensor_scalar_add(vt0[:], vt0[:], 1.0)
            nc.vector.reciprocal(vt1[:], vt0[:])
            nc.vector.tensor_tensor(vt1[:], vt1[:], reach[:], op=ALU.mult)
            nc.vector.tensor_scalar_mul(g_v[:], vt1[:], 16.0)
            nc.sync.dma_start(g_d[:], g_v[:])
            nc.sync.dma_start(g_b[:], g_d[0:1, :].to_broadcast((128, B)))

            # ---- P4: cos-norm classifier ----
            with tc.tile_pool(name="p4", bufs=1) as p4, \
                 tc.tile_pool(name="p4ps", bufs=1, space="PSUM") as p4ps:
                for rc in range(RC):
                    lg_ps = [p4ps.tile([CP, RW], f32, tag=f"lg{ct}", name=f"lg{ct}")
                             for ct in range(CT)]
                    for k in range(KT):
                        wcosk = pp.tile([128, C], f32r, tag="wcosk", bufs=2)
                        nc.sync.dma_start(wcosk[:], WCT[k * 128:(k + 1) * 128, :])
                        h2k = pp.tile([128, RW], f32r, tag="h2k", bufs=3)
                        nc.sync.dma_start(
                            h2k[:],
                            h2s[k * 128:(k + 1) * 128, rc * RW:(rc + 1) * RW])
                        for ct in range(CT):
                            nc.tensor.matmul(
                                lg_ps[ct][:],
                                wcosk[:, ct * CP:(ct + 1) * CP],
                                h2k[:],
                                start=(k == 0), stop=(k == KT - 1))
                    for ct in range(CT):
                        lgout = pp.tile([CP, RW], f32, tag="lgout", bufs=2)
                        nc.vector.tensor_tensor(
                            lgout[:], lg_ps[ct][:],
                            g_b[0:CP, rc * RW:(rc + 1) * RW], op=ALU.mult)
                        nc.sync.dma_start(
                            LG[ct * CP:(ct + 1) * CP, rc * RW:(rc + 1) * RW],
                            lgout[:])

    nc.compile()
    return nc


def kernel(x, centroids, W_hall, b_hall, W_sel, b_sel, W_cos):
    x = np.asarray(x, np.float32)
    centroids = np.asarray(centroids, np.float32)
    W_hall = np.asarray(W_hall, np.float32)
    b_hall = np.asarray(b_hall, np.float32)
    W_sel = np.asarray(W_sel, np.float32)
    b_sel = np.asarray(b_sel, np.float32)
    W_cos = np.asarray(W_cos, np.float32)

    if "nc" not in _CACHE:
        _CACHE["nc"] = _build_program()
    nc = _CACHE["nc"]

    NB = x.shape[0]
    ncores = 8
    shard = NB // ncores

    wnorm = W_cos / np.linalg.norm(W_cos, axis=1, keepdims=True)
    shared = {
        "centT": np.ascontiguousarray(centroids.T),
        "centN": np.ascontiguousarray(centroids),
        "whallT": np.ascontiguousarray(W_hall.T),
        "wselT": np.ascontiguousarray(W_sel.T),
        "wcosnT": np.ascontiguousarray(wnorm.T),
        "bhall": np.ascontiguousarray(b_hall.reshape(C, 1)),
        "bsel": np.ascontiguousarray(b_sel.reshape(D, 1)),
        "ncsq": np.ascontiguousarray((-np.sum(centroids * centroids, axis=1) / 2.0)
                                     .reshape(1, C)),
        "onesc": np.ones((128, 1), np.float32),
        "onesr": np.ones((1, 128), np.float32),
    }
    in_maps = []
    for i in range(ncores):
        m = dict(shared)
        m["xT"] = np.ascontiguousarray(x[i * shard:(i + 1) * shard].T)
        in_maps.append(m)

    res = run_bass_kernel_spmd(nc, in_maps, list(range(ncores)), **_CACHE.get("run_kwargs", {}))
    _CACHE["last_result"] = res

    logits = np.concatenate(
        [np.ascontiguousarray(res.results[i]["logitsT"].T) for i in range(ncores)],
        axis=0)
    infused = np.concatenate(
        [np.ascontiguousarray(res.results[i]["infusedT"].T) for i in range(ncores)],
        axis=0)
    return logits, x, infused
